# revision 12
# baseline (speedup 1.0000x reference)
"""2-layer GAT on 8 trn2 NeuronCores.

Strategy (self-contained, hardcoded for N=100000, E=3200000, 128->64->32):
 - Host does index prep + data layout only (degree-sort, dst-block packing,
   per-edge expansion of device-computed tables via np.take, concat/unshard).
   All model math (matmuls, attention, softmax) runs on device.
 - prog0: node-sharded dense table build H1 = [x@W1 | x@W1@a_s | x@W1@a_d]
   (each core computes N/8 rows).
 - host: expand H1 rows into per-edge dst-major block layout (the "gather"
   permutation is host-known index movement).
 - prog1: stream per-edge rows with direct DMA; per dst-block (128 dsts on
   partitions, K edge slots along free dim) segment softmax + weighted mean
   fully on-chip; project to layer-2 table rows.
 - host: reassemble layer-2 table by node, expand per-edge again.
 - prog2: same aggregation for layer 2 + final row softmax.
"""

import sys
from contextlib import ExitStack

import numpy as np

sys.path.insert(0, "/opt/trn_rl_repo")

import ml_dtypes  # noqa: E402

import concourse.bass as bass  # noqa: E402
import concourse.bacc as bacc  # noqa: E402
import concourse.tile as tile  # noqa: E402
from concourse import mybir  # noqa: E402
from concourse.bass_utils import run_bass_kernel_spmd  # noqa: E402
from concourse.masks import make_identity  # noqa: E402

N = 100000
E = 3200000
IN_F, HID_F, OUT_F = 128, 64, 32
NEG = 0.2
CORES = 8
P = 128
NBLK = 98            # per-core dst blocks
NPC = NBLK * P       # 12544 per-core node slots
NSH = N // CORES     # 12500 table rows built per core in prog0
SENT = N             # sentinel row id (gs=gd=-1e30 -> exp()=0)
E1 = HID_F + 2       # 66 bf16 elems per layer-1 row: h(64) | gs | gd
E2 = OUT_F + 2       # 34 bf16 elems per layer-2 row

bf = mybir.dt.bfloat16
f32 = mybir.dt.float32
AF = mybir.ActivationFunctionType
OP = mybir.AluOpType

LAST_RESULT = None
_CACHE = {}


# ----------------------------------------------------------------- host prep
def _host_prep(edge_index):
    src = np.asarray(edge_index[0], dtype=np.int64)
    dst = np.asarray(edge_index[1], dtype=np.int64)
    deg = np.bincount(dst, minlength=N).astype(np.int64) + 1  # incl self loop
    order = np.argsort(-deg, kind="stable")                   # global pos -> node
    degs = deg[order]
    Ks = [int(degs[j * CORES * P]) for j in range(NBLK)]

    # edges grouped by dst
    eorder = np.argsort(dst, kind="stable")
    ssorted = src[eorder]
    dsorted = dst[eorder]
    counts = np.bincount(dst, minlength=N)
    starts = np.zeros(N, dtype=np.int64)
    starts[1:] = np.cumsum(counts)[:-1]

    pos_of_node = np.empty(N, dtype=np.int64)                 # node -> global pos
    pos_of_node[order] = np.arange(N)

    GSLOTS = NBLK * CORES * P  # 100352
    Kmax = max(Ks)
    M = np.full((GSLOTS, Kmax), SENT, dtype=np.int32)
    M[:N, 0] = order.astype(np.int32)                          # self loop at k=0
    slot_k = (np.arange(E) - starts[dsorted] + 1).astype(np.int64)
    M[pos_of_node[dsorted], slot_k] = ssorted.astype(np.int32)

    TOT = P * sum(Ks)
    idx1 = np.empty((CORES, TOT), dtype=np.int32)
    for c in range(CORES):
        off = 0
        for j in range(NBLK):
            g0 = (j * CORES + c) * P
            K = Ks[j]
            idx1[c, off:off + P * K] = M[g0:g0 + P, :K].reshape(-1)
            off += P * K
    return Ks, order, idx1


# ------------------------------------------------------------- device programs
def _build_nc0():
    """Node-sharded table build: h1s = [x@W1 | gs | gd] for N/8 nodes."""
    nc = bacc.Bacc("TRN2", target_bir_lowering=False, debug=False,
                   enable_asserts=False, num_devices=CORES)
    xTs = nc.dram_tensor("xts", [IN_F, NSH], bf, kind="ExternalInput")
    w1e = nc.dram_tensor("w1e", [IN_F, E1], bf, kind="ExternalInput")
    h1s = nc.dram_tensor("h1s", [NSH, E1], bf, kind="ExternalOutput")

    with ExitStack() as ctx:
        tc = ctx.enter_context(tile.TileContext(nc))
        consts = ctx.enter_context(tc.tile_pool(name="consts", bufs=1))
        psum = ctx.enter_context(tc.tile_pool(name="psum", bufs=4, space="PSUM"))
        sb = ctx.enter_context(tc.tile_pool(name="sb", bufs=3))
        w1sb = consts.tile([IN_F, E1], bf)
        nc.sync.dma_start(out=w1sb[:], in_=w1e.ap())

        CH = 2048
        NB = (NSH + CH - 1) // CH
        for gq in range(NB):
            n0 = gq * CH
            nn = min(CH, NSH - n0)
            nq_full = nn // P
            xt_t = sb.tile([IN_F, CH], bf, tag="xt")
            nc.sync.dma_start(out=xt_t[:, :nn], in_=xTs.ap()[:, n0:n0 + nn])
            tb = sb.tile([P, (CH // P) * E1], bf, tag="tb")
            for q in range(nq_full):
                p66 = psum.tile([P, E1], f32, tag="p66")
                nc.tensor.matmul(out=p66[:], lhsT=xt_t[:, q * P:(q + 1) * P],
                                 rhs=w1sb[:], start=True, stop=True)
                nc.scalar.activation(out=tb[:, q * E1:(q + 1) * E1],
                                     in_=p66[:], func=AF.Copy)
            if nq_full:
                nc.sync.dma_start(
                    out=h1s.ap()[n0:n0 + nq_full * P, :].rearrange(
                        "(q p) e -> p q e", p=P),
                    in_=tb[:, :nq_full * E1].rearrange("p (q e) -> p q e", e=E1))
            if nn % P:
                q = nq_full
                qa = nn % P
                p66 = psum.tile([P, E1], f32, tag="p66")
                nc.tensor.matmul(out=p66[:qa, :],
                                 lhsT=xt_t[:, q * P:q * P + qa],
                                 rhs=w1sb[:], start=True, stop=True)
                tbr = sb.tile([P, E1], bf, tag="tbr")
                nc.scalar.activation(out=tbr[:qa, :], in_=p66[:qa, :],
                                     func=AF.Copy)
                nc.sync.dma_start(out=h1s.ap()[n0 + q * P:n0 + nn, :],
                                  in_=tbr[:qa, :])
    nc.compile()
    return nc


def _agg_layer(nc, sb, psum, Ks, he, ew, fw, brep, wnext, h2l, ident, outp):
    K0 = Ks[0]
    off = 0
    for j in range(NBLK):
        K = Ks[j]
        # stream the host-expanded per-edge rows: hg[p, k*ew:(k+1)*ew] is the
        # k-th edge row of dst slot p of this block
        hg = sb.tile([P, K * ew], bf, tag="hg", padded_shape=[P, K0 * ew])
        nc.sync.dma_start(
            out=hg[:],
            in_=he.ap()[off:off + P * K, :].rearrange("(p k) e -> p (k e)", p=P))
        hg3 = hg[:].rearrange("p (k e) -> p k e", e=ew)
        # logits: z = gs_src + gd_dst (gd from the k=0 self-loop row);
        # both ops on ACT (Identity/Copy live in every table set)
        gdf = sb.tile([P, 1], f32, tag="gdf")
        nc.scalar.activation(out=gdf[:], in_=hg[:, fw + 1:fw + 2], func=AF.Copy)
        z = sb.tile([P, K], f32, tag="z")
        nc.scalar.activation(
            out=z[:],
            in_=hg3[:, :, fw:fw + 1].rearrange("p k o -> p (k o)"),
            func=AF.Identity, bias=gdf[:])
        zl = sb.tile([P, K], bf, tag="zl")
        nc.vector.scalar_tensor_tensor(out=zl[:], in0=z[:], scalar=NEG,
                                       in1=z[:], op0=OP.mult, op1=OP.max)
        ez = sb.tile([P, K], bf, tag="ez")
        den = sb.tile([P, 1], f32, tag="den")
        nc.scalar.activation(out=ez[:], in_=zl[:], func=AF.Exp,
                             accum_out=den[:])
        r = sb.tile([P, 1], f32, tag="r")
        nc.vector.reciprocal(out=r[:], in_=den[:])
        # weighted rows: tmp[p, k, j] = h[p, k, j] * ez[p, k]  (unit-stride)
        tmp = sb.tile([P, K * fw], bf, tag="tmp", padded_shape=[P, K0 * fw])
        nc.vector.tensor_tensor(
            out=tmp[:].rearrange("p (k j) -> p k j", j=fw),
            in0=hg3[:, :, 0:fw],
            in1=ez[:].rearrange("p (k o) -> p k o", o=1).to_broadcast([P, K, fw]),
            op=OP.mult)
        # num[p, j] = sum_k tmp[p, k, j]: unit-stride folding tree (bf16),
        # final fold in f32
        m = K
        while m > 2:
            h = m // 2
            nc.vector.tensor_tensor(
                out=tmp[:, 0:h * fw], in0=tmp[:, 0:h * fw],
                in1=tmp[:, (m - h) * fw:m * fw], op=OP.add)
            m -= h
        num = sb.tile([P, fw], f32, tag="num")
        nc.vector.tensor_tensor(out=num[:], in0=tmp[:, 0:fw],
                                in1=tmp[:, fw:2 * fw], op=OP.add)
        o1 = sb.tile([P, fw], f32, tag="o1")
        nc.vector.scalar_tensor_tensor(out=o1[:], in0=num[:], scalar=r[:],
                                       in1=brep[:], op0=OP.mult, op1=OP.add)
        if wnext is not None:
            o1b = sb.tile([P, fw], bf, tag="o1b")
            nc.scalar.activation(out=o1b[:], in_=o1[:], func=AF.Relu)
            pt = psum.tile([fw, P], bf, tag="pt")
            nc.tensor.transpose(out=pt[:], in_=o1b[:], identity=ident[:])
            o1T = sb.tile([fw, P], bf, tag="o1T")
            nc.scalar.activation(out=o1T[:], in_=pt[:], func=AF.Copy)
            p34 = psum.tile([P, E2], f32, tag="p34")
            nc.tensor.matmul(out=p34[:], lhsT=o1T[:], rhs=wnext[:],
                             start=True, stop=True)
            th2 = sb.tile([P, E2], bf, tag="th2")
            nc.scalar.activation(out=th2[:], in_=p34[:], func=AF.Copy)
            nc.sync.dma_start(out=h2l.ap()[j * P:(j + 1) * P, :], in_=th2[:])
        else:
            # final row softmax; logits are O(5) so no max-subtraction needed
            e2 = sb.tile([P, fw], f32, tag="e2")
            ssum = sb.tile([P, 1], f32, tag="ssum")
            nc.scalar.activation(out=e2[:], in_=o1[:], func=AF.Exp,
                                 accum_out=ssum[:])
            rs = sb.tile([P, 1], f32, tag="rs")
            nc.vector.reciprocal(out=rs[:], in_=ssum[:])
            of = sb.tile([P, fw], f32, tag="of")
            nc.scalar.activation(out=of[:], in_=e2[:], func=AF.Copy,
                                 scale=rs[:])
            nc.sync.dma_start(out=outp.ap()[j * P:(j + 1) * P, :], in_=of[:])
        off += P * K


def _mk_agg_consts(nc, tc, ctx, bd, fw, w2e):
    consts = ctx.enter_context(tc.tile_pool(name="consts", bufs=1))
    psum = ctx.enter_context(tc.tile_pool(name="psum", bufs=2, space="PSUM"))
    out = {"psum": psum}
    ident = consts.tile([P, P], bf)
    make_identity(nc, ident[:])
    out["ident"] = ident
    ones1 = consts.tile([1, P], bf)
    nc.gpsimd.memset(ones1[:], 1.0)
    br = consts.tile([1, fw], bf)
    nc.sync.dma_start(out=br[:], in_=bd.ap())
    brep = consts.tile([P, fw], f32)
    pb = psum.tile([P, fw], f32, tag="pb")
    nc.tensor.matmul(out=pb[:], lhsT=ones1[:], rhs=br[:], start=True, stop=True)
    nc.vector.tensor_copy(out=brep[:], in_=pb[:])
    out["brep"] = brep
    if w2e is not None:
        w2sb = consts.tile([HID_F, E2], bf)
        nc.sync.dma_start(out=w2sb[:], in_=w2e.ap())
        out["w2sb"] = w2sb
    return out


def _build_nc1(Ks):
    TOT = P * sum(Ks)
    nc = bacc.Bacc("TRN2", target_bir_lowering=False, debug=False,
                   enable_asserts=False, num_devices=CORES)
    he1 = nc.dram_tensor("he1", [TOT, E1], bf, kind="ExternalInput")
    w2e = nc.dram_tensor("w2e", [HID_F, E2], bf, kind="ExternalInput")
    b1d = nc.dram_tensor("b1d", [1, HID_F], bf, kind="ExternalInput")
    h2lo = nc.dram_tensor("h2lo", [NPC, E2], bf, kind="ExternalOutput")

    with ExitStack() as ctx:
        tc = ctx.enter_context(tile.TileContext(nc))
        cc = _mk_agg_consts(nc, tc, ctx, b1d, HID_F, w2e)
        sb = ctx.enter_context(tc.tile_pool(name="sb", bufs=3))
        _agg_layer(nc, sb, cc["psum"], Ks, he1, E1, HID_F, cc["brep"],
                   cc["w2sb"], h2lo, cc["ident"], None)
    nc.compile()
    return nc


def _build_nc2(Ks):
    TOT = P * sum(Ks)
    nc = bacc.Bacc("TRN2", target_bir_lowering=False, debug=False,
                   enable_asserts=False, num_devices=CORES)
    he2 = nc.dram_tensor("he2", [TOT, E2], bf, kind="ExternalInput")
    b2d = nc.dram_tensor("b2d", [1, OUT_F], bf, kind="ExternalInput")
    outp = nc.dram_tensor("outp", [NPC, OUT_F], f32, kind="ExternalOutput")

    with ExitStack() as ctx:
        tc = ctx.enter_context(tile.TileContext(nc))
        cc = _mk_agg_consts(nc, tc, ctx, b2d, OUT_F, None)
        sb = ctx.enter_context(tc.tile_pool(name="sb", bufs=3))
        _agg_layer(nc, sb, cc["psum"], Ks, he2, E2, OUT_F, cc["brep"],
                   None, None, cc["ident"], outp)
    nc.compile()
    return nc


# ------------------------------------------------------------------- kernel
def kernel(x, edge_index, W1, att_src1, att_dst1, b1, W2, att_src2, att_dst2,
           b2, _trace=False):
    global LAST_RESULT
    bfnp = ml_dtypes.bfloat16
    x = np.asarray(x, dtype=np.float32)
    W1 = np.asarray(W1, dtype=np.float32)
    W2 = np.asarray(W2, dtype=np.float32)

    Ks, order, idx1 = _host_prep(np.asarray(edge_index))

    key = tuple(Ks)
    if key not in _CACHE:
        _CACHE[key] = (_build_nc0(), _build_nc1(Ks), _build_nc2(Ks))
    nc0, nc1, nc2 = _CACHE[key]

    xT = np.ascontiguousarray(x.T).astype(bfnp)
    w1ext = np.concatenate(
        [W1, (W1 @ np.asarray(att_src1, np.float32))[:, None],
         (W1 @ np.asarray(att_dst1, np.float32))[:, None]], axis=1).astype(bfnp)
    w2ext = np.concatenate(
        [W2, (W2 @ np.asarray(att_src2, np.float32))[:, None],
         (W2 @ np.asarray(att_dst2, np.float32))[:, None]], axis=1).astype(bfnp)
    b1a = np.asarray(b1, np.float32)[None, :].astype(bfnp)
    b2a = np.asarray(b2, np.float32)[None, :].astype(bfnp)

    # prog0: node-sharded table build
    in0 = [{"xts": np.ascontiguousarray(xT[:, c * NSH:(c + 1) * NSH]),
            "w1e": w1ext} for c in range(CORES)]
    r0 = run_bass_kernel_spmd(nc0, in0, core_ids=list(range(CORES)),
                              trace=_trace)
    H1cat = np.empty((N + 1, E1), dtype=bfnp)
    for c in range(CORES):
        H1cat[c * NSH:(c + 1) * NSH] = np.asarray(r0.results[c]["h1s"]).reshape(NSH, E1)
    H1cat[N] = bfnp(0.0)
    H1cat[N, HID_F:] = bfnp(-1e30)

    # host expansion: per-edge dst-major rows (index movement only)
    in1 = [{"he1": H1cat[idx1[c]], "w2e": w2ext, "b1d": b1a}
           for c in range(CORES)]
    r1 = run_bass_kernel_spmd(nc1, in1, core_ids=list(range(CORES)),
                              trace=_trace)

    # reassemble layer-2 table by node id, then expand per-edge again
    h2n = np.empty((N + 1, E2), dtype=bfnp)
    pp = np.arange(P)
    jj = np.arange(NBLK)
    for c in range(CORES):
        oc = np.asarray(r1.results[c]["h2lo"]).reshape(NPC, E2)
        g = ((jj * CORES + c)[:, None] * P + pp[None, :]).reshape(-1)
        valid = g < N
        h2n[order[g[valid]]] = oc[valid]
    h2n[N] = bfnp(0.0)
    h2n[N, OUT_F:] = bfnp(-1e30)

    in2 = [{"he2": h2n[idx1[c]], "b2d": b2a} for c in range(CORES)]
    r2 = run_bass_kernel_spmd(nc2, in2, core_ids=list(range(CORES)),
                              trace=_trace)
    LAST_RESULT = (r0, r1, r2)

    out = np.zeros((N, OUT_F), dtype=np.float32)
    for c in range(CORES):
        oc = np.asarray(r2.results[c]["outp"]).reshape(NPC, OUT_F)
        g = ((jj * CORES + c)[:, None] * P + pp[None, :]).reshape(-1)
        valid = g < N
        out[order[g[valid]]] = oc[valid]
    return out


# revision 16
# speedup vs baseline: 1.1238x; 1.1238x over previous
"""2-layer GAT on 8 trn2 NeuronCores.

Strategy (self-contained, hardcoded for N=100000, E=3200000, 128->64->32):
 - Host does index prep + data layout only (degree-sort, dst-block packing,
   per-edge expansion of device-computed tables via np.take, concat/unshard).
   All model math (matmuls, attention, softmax) runs on device.
 - prog0: node-sharded dense table build H1 = [x@W1 | x@W1@a_s | x@W1@a_d]
   (each core computes N/8 rows).
 - host: expand H1 rows into per-edge dst-major block layout (the "gather"
   permutation is host-known index movement).
 - prog1: stream per-edge rows with direct DMA; per dst-block (128 dsts on
   partitions, K edge slots along free dim) segment softmax + weighted mean
   fully on-chip; project to layer-2 table rows.
 - host: reassemble layer-2 table by node, expand per-edge again.
 - prog2: same aggregation for layer 2 + final row softmax.
"""

import sys
from contextlib import ExitStack

import numpy as np

sys.path.insert(0, "/opt/trn_rl_repo")

import ml_dtypes  # noqa: E402

import concourse.bass as bass  # noqa: E402
import concourse.bacc as bacc  # noqa: E402
import concourse.tile as tile  # noqa: E402
from concourse import mybir  # noqa: E402
from concourse.bass_utils import run_bass_kernel_spmd  # noqa: E402
from concourse.masks import make_identity  # noqa: E402

N = 100000
E = 3200000
IN_F, HID_F, OUT_F = 128, 64, 32
NEG = 0.2
CORES = 8
P = 128
NBLK = 98            # per-core dst blocks
NPC = NBLK * P       # 12544 per-core node slots
NSH = N // CORES     # 12500 table rows built per core in prog0
SENT = N             # sentinel row id (gs=gd=-1e30 -> exp()=0)
E1 = HID_F + 2       # 66 bf16 elems per layer-1 row: h(64) | gs | gd
E2 = OUT_F + 2       # 34 bf16 elems per layer-2 row

bf = mybir.dt.bfloat16
f32 = mybir.dt.float32
AF = mybir.ActivationFunctionType
OP = mybir.AluOpType

LAST_RESULT = None
_CACHE = {}


# ----------------------------------------------------------------- host prep
def _host_prep(edge_index):
    src = np.asarray(edge_index[0], dtype=np.int64)
    dst = np.asarray(edge_index[1], dtype=np.int64)
    deg = np.bincount(dst, minlength=N).astype(np.int64) + 1  # incl self loop
    order = np.argsort(-deg, kind="stable")                   # global pos -> node
    degs = deg[order]
    Ks = [int(degs[j * CORES * P]) for j in range(NBLK)]

    # edges grouped by dst
    eorder = np.argsort(dst, kind="stable")
    ssorted = src[eorder]
    dsorted = dst[eorder]
    counts = np.bincount(dst, minlength=N)
    starts = np.zeros(N, dtype=np.int64)
    starts[1:] = np.cumsum(counts)[:-1]

    pos_of_node = np.empty(N, dtype=np.int64)                 # node -> global pos
    pos_of_node[order] = np.arange(N)

    GSLOTS = NBLK * CORES * P  # 100352
    Kmax = max(Ks)
    M = np.full((GSLOTS, Kmax), SENT, dtype=np.int32)
    M[:N, 0] = order.astype(np.int32)                          # self loop at k=0
    slot_k = (np.arange(E) - starts[dsorted] + 1).astype(np.int64)
    M[pos_of_node[dsorted], slot_k] = ssorted.astype(np.int32)

    TOT = P * sum(Ks)
    idx1 = np.empty((CORES, TOT), dtype=np.int32)
    for c in range(CORES):
        off = 0
        for j in range(NBLK):
            g0 = (j * CORES + c) * P
            K = Ks[j]
            idx1[c, off:off + P * K] = M[g0:g0 + P, :K].reshape(-1)
            off += P * K
    return Ks, order, idx1


# ------------------------------------------------------------- device programs
def _build_nc0():
    """Node-sharded table build: h1s = [x@W1 | gs | gd] for N/8 nodes."""
    nc = bacc.Bacc("TRN2", target_bir_lowering=False, debug=False,
                   enable_asserts=False, num_devices=CORES)
    xTs = nc.dram_tensor("xts", [IN_F, NSH], bf, kind="ExternalInput")
    w1e = nc.dram_tensor("w1e", [IN_F, E1], bf, kind="ExternalInput")
    h1s = nc.dram_tensor("h1s", [NSH, E1], bf, kind="ExternalOutput")

    with ExitStack() as ctx:
        tc = ctx.enter_context(tile.TileContext(nc))
        consts = ctx.enter_context(tc.tile_pool(name="consts", bufs=1))
        psum = ctx.enter_context(tc.tile_pool(name="psum", bufs=4, space="PSUM"))
        sb = ctx.enter_context(tc.tile_pool(name="sb", bufs=3))
        w1sb = consts.tile([IN_F, E1], bf)
        nc.sync.dma_start(out=w1sb[:], in_=w1e.ap())

        CH = 2048
        NB = (NSH + CH - 1) // CH
        for gq in range(NB):
            n0 = gq * CH
            nn = min(CH, NSH - n0)
            nq_full = nn // P
            xt_t = sb.tile([IN_F, CH], bf, tag="xt")
            nc.sync.dma_start(out=xt_t[:, :nn], in_=xTs.ap()[:, n0:n0 + nn])
            tb = sb.tile([P, (CH // P) * E1], bf, tag="tb")
            for q in range(nq_full):
                p66 = psum.tile([P, E1], f32, tag="p66")
                nc.tensor.matmul(out=p66[:], lhsT=xt_t[:, q * P:(q + 1) * P],
                                 rhs=w1sb[:], start=True, stop=True)
                nc.scalar.activation(out=tb[:, q * E1:(q + 1) * E1],
                                     in_=p66[:], func=AF.Copy)
            if nq_full:
                nc.sync.dma_start(
                    out=h1s.ap()[n0:n0 + nq_full * P, :].rearrange(
                        "(q p) e -> p q e", p=P),
                    in_=tb[:, :nq_full * E1].rearrange("p (q e) -> p q e", e=E1))
            if nn % P:
                q = nq_full
                qa = nn % P
                p66 = psum.tile([P, E1], f32, tag="p66")
                nc.tensor.matmul(out=p66[:qa, :],
                                 lhsT=xt_t[:, q * P:q * P + qa],
                                 rhs=w1sb[:], start=True, stop=True)
                tbr = sb.tile([P, E1], bf, tag="tbr")
                nc.scalar.activation(out=tbr[:qa, :], in_=p66[:qa, :],
                                     func=AF.Copy)
                nc.sync.dma_start(out=h1s.ap()[n0 + q * P:n0 + nn, :],
                                  in_=tbr[:qa, :])
    nc.compile()
    return nc


def _agg_layer(nc, sb, psum, Ks, he, ew, fw, brep, wnext, h2l, ident, outp):
    K0 = Ks[0]
    off = 0
    for j in range(NBLK):
        K = Ks[j]
        # stream the host-expanded per-edge rows: hg[p, k*ew:(k+1)*ew] is the
        # k-th edge row of dst slot p of this block
        hg = sb.tile([P, K * ew], bf, tag="hg", padded_shape=[P, K0 * ew])
        nc.sync.dma_start(
            out=hg[:],
            in_=he.ap()[off:off + P * K, :].rearrange("(p k) e -> p (k e)", p=P))
        hg3 = hg[:].rearrange("p (k e) -> p k e", e=ew)
        # logits: z = gs_src + gd_dst (gd from the k=0 self-loop row);
        # both ops on ACT (Identity/Copy live in every table set)
        gdf = sb.tile([P, 1], f32, tag="gdf")
        nc.scalar.activation(out=gdf[:], in_=hg[:, fw + 1:fw + 2], func=AF.Copy)
        z = sb.tile([P, K], f32, tag="z")
        nc.scalar.activation(
            out=z[:],
            in_=hg3[:, :, fw:fw + 1].rearrange("p k o -> p (k o)"),
            func=AF.Identity, bias=gdf[:])
        zl = sb.tile([P, K], bf, tag="zl")
        nc.vector.scalar_tensor_tensor(out=zl[:], in0=z[:], scalar=NEG,
                                       in1=z[:], op0=OP.mult, op1=OP.max)
        ez = sb.tile([P, K], bf, tag="ez")
        den = sb.tile([P, 1], f32, tag="den")
        nc.scalar.activation(out=ez[:], in_=zl[:], func=AF.Exp,
                             accum_out=den[:])
        r = sb.tile([P, 1], f32, tag="r")
        nc.vector.reciprocal(out=r[:], in_=den[:])
        # weighted rows: tmp[p, k, j] = h[p, k, j] * ez[p, k]  (unit-stride)
        tmp = sb.tile([P, K * fw], bf, tag="tmp", padded_shape=[P, K0 * fw])
        nc.vector.tensor_tensor(
            out=tmp[:].rearrange("p (k j) -> p k j", j=fw),
            in0=hg3[:, :, 0:fw],
            in1=ez[:].rearrange("p (k o) -> p k o", o=1).to_broadcast([P, K, fw]),
            op=OP.mult)
        # num[p, j] = sum_k tmp[p, k, j]: unit-stride folding tree (bf16),
        # final fold in f32
        m = K
        while m > 2:
            h = m // 2
            nc.vector.tensor_tensor(
                out=tmp[:, 0:h * fw], in0=tmp[:, 0:h * fw],
                in1=tmp[:, (m - h) * fw:m * fw], op=OP.add)
            m -= h
        num = sb.tile([P, fw], f32, tag="num")
        nc.vector.tensor_tensor(out=num[:], in0=tmp[:, 0:fw],
                                in1=tmp[:, fw:2 * fw], op=OP.add)
        o1 = sb.tile([P, fw], f32, tag="o1")
        nc.vector.scalar_tensor_tensor(out=o1[:], in0=num[:], scalar=r[:],
                                       in1=brep[:], op0=OP.mult, op1=OP.add)
        if wnext is not None:
            o1b = sb.tile([P, fw], bf, tag="o1b")
            nc.scalar.activation(out=o1b[:], in_=o1[:], func=AF.Relu)
            pt = psum.tile([fw, P], bf, tag="pt")
            nc.tensor.transpose(out=pt[:], in_=o1b[:], identity=ident[:])
            o1T = sb.tile([fw, P], bf, tag="o1T")
            nc.scalar.activation(out=o1T[:], in_=pt[:], func=AF.Copy)
            p34 = psum.tile([P, E2], f32, tag="p34")
            nc.tensor.matmul(out=p34[:], lhsT=o1T[:], rhs=wnext[:],
                             start=True, stop=True)
            th2 = sb.tile([P, E2], bf, tag="th2")
            nc.scalar.activation(out=th2[:], in_=p34[:], func=AF.Copy)
            nc.sync.dma_start(out=h2l.ap()[j * P:(j + 1) * P, :], in_=th2[:])
        else:
            # final row softmax; logits are O(5) so no max-subtraction needed
            e2 = sb.tile([P, fw], f32, tag="e2")
            ssum = sb.tile([P, 1], f32, tag="ssum")
            nc.scalar.activation(out=e2[:], in_=o1[:], func=AF.Exp,
                                 accum_out=ssum[:])
            rs = sb.tile([P, 1], f32, tag="rs")
            nc.vector.reciprocal(out=rs[:], in_=ssum[:])
            of = sb.tile([P, fw], f32, tag="of")
            nc.scalar.activation(out=of[:], in_=e2[:], func=AF.Copy,
                                 scale=rs[:])
            nc.sync.dma_start(out=outp.ap()[j * P:(j + 1) * P, :], in_=of[:])
        off += P * K


def _mk_agg_consts(nc, tc, ctx, bd, fw, w2e):
    consts = ctx.enter_context(tc.tile_pool(name="consts", bufs=1))
    psum = ctx.enter_context(tc.tile_pool(name="psum", bufs=2, space="PSUM"))
    out = {"psum": psum}
    ident = consts.tile([P, P], bf)
    make_identity(nc, ident[:])
    out["ident"] = ident
    ones1 = consts.tile([1, P], bf)
    nc.gpsimd.memset(ones1[:], 1.0)
    br = consts.tile([1, fw], bf)
    nc.sync.dma_start(out=br[:], in_=bd.ap())
    brep = consts.tile([P, fw], f32)
    pb = psum.tile([P, fw], f32, tag="pb")
    nc.tensor.matmul(out=pb[:], lhsT=ones1[:], rhs=br[:], start=True, stop=True)
    nc.vector.tensor_copy(out=brep[:], in_=pb[:])
    out["brep"] = brep
    if w2e is not None:
        w2sb = consts.tile([HID_F, E2], bf)
        nc.sync.dma_start(out=w2sb[:], in_=w2e.ap())
        out["w2sb"] = w2sb
    return out


def _build_nc1(Ks):
    TOT = P * sum(Ks)
    nc = bacc.Bacc("TRN2", target_bir_lowering=False, debug=False,
                   enable_asserts=False, num_devices=CORES)
    he1 = nc.dram_tensor("he1", [TOT, E1], bf, kind="ExternalInput")
    w2e = nc.dram_tensor("w2e", [HID_F, E2], bf, kind="ExternalInput")
    b1d = nc.dram_tensor("b1d", [1, HID_F], bf, kind="ExternalInput")
    h2lo = nc.dram_tensor("h2lo", [NPC, E2], bf, kind="ExternalOutput")

    with ExitStack() as ctx:
        tc = ctx.enter_context(tile.TileContext(nc))
        cc = _mk_agg_consts(nc, tc, ctx, b1d, HID_F, w2e)
        sb = ctx.enter_context(tc.tile_pool(name="sb", bufs=3))
        _agg_layer(nc, sb, cc["psum"], Ks, he1, E1, HID_F, cc["brep"],
                   cc["w2sb"], h2lo, cc["ident"], None)
    nc.compile()
    return nc


def _build_nc2(Ks):
    TOT = P * sum(Ks)
    nc = bacc.Bacc("TRN2", target_bir_lowering=False, debug=False,
                   enable_asserts=False, num_devices=CORES)
    he2 = nc.dram_tensor("he2", [TOT, E2], bf, kind="ExternalInput")
    b2d = nc.dram_tensor("b2d", [1, OUT_F], bf, kind="ExternalInput")
    outp = nc.dram_tensor("outp", [NPC, OUT_F], f32, kind="ExternalOutput")

    with ExitStack() as ctx:
        tc = ctx.enter_context(tile.TileContext(nc))
        cc = _mk_agg_consts(nc, tc, ctx, b2d, OUT_F, None)
        sb = ctx.enter_context(tc.tile_pool(name="sb", bufs=3))
        _agg_layer(nc, sb, cc["psum"], Ks, he2, E2, OUT_F, cc["brep"],
                   None, None, cc["ident"], outp)
    nc.compile()
    return nc


# ------------------------------------------------------------------- kernel
def kernel(x, edge_index, W1, att_src1, att_dst1, b1, W2, att_src2, att_dst2,
           b2, _trace=False):
    global LAST_RESULT
    bfnp = ml_dtypes.bfloat16
    x = np.asarray(x, dtype=np.float32)
    W1 = np.asarray(W1, dtype=np.float32)
    W2 = np.asarray(W2, dtype=np.float32)

    Ks, order, idx1 = _host_prep(np.asarray(edge_index))

    key = tuple(Ks)
    if key not in _CACHE:
        _CACHE[key] = (_build_nc0(), _build_nc1(Ks), _build_nc2(Ks))
    nc0, nc1, nc2 = _CACHE[key]

    xT = np.ascontiguousarray(x.T).astype(bfnp)
    w1ext = np.concatenate(
        [W1, (W1 @ np.asarray(att_src1, np.float32))[:, None],
         (W1 @ np.asarray(att_dst1, np.float32))[:, None]], axis=1).astype(bfnp)
    w2ext = np.concatenate(
        [W2, (W2 @ np.asarray(att_src2, np.float32))[:, None],
         (W2 @ np.asarray(att_dst2, np.float32))[:, None]], axis=1).astype(bfnp)
    b1a = np.asarray(b1, np.float32)[None, :].astype(bfnp)
    b2a = np.asarray(b2, np.float32)[None, :].astype(bfnp)

    # prog0: node-sharded table build
    in0 = [{"xts": np.ascontiguousarray(xT[:, c * NSH:(c + 1) * NSH]),
            "w1e": w1ext} for c in range(CORES)]
    r0 = run_bass_kernel_spmd(nc0, in0, core_ids=list(range(CORES)),
                              trace=_trace)
    H1cat = np.empty((N + 1, E1), dtype=bfnp)
    for c in range(CORES):
        H1cat[c * NSH:(c + 1) * NSH] = np.asarray(r0.results[c]["h1s"]).reshape(NSH, E1)
    H1cat[N] = bfnp(0.0)
    H1cat[N, HID_F:] = bfnp(-1e30)

    # host expansion: per-edge dst-major rows (index movement only)
    in1 = [{"he1": H1cat[idx1[c]], "w2e": w2ext, "b1d": b1a}
           for c in range(CORES)]
    r1 = run_bass_kernel_spmd(nc1, in1, core_ids=list(range(CORES)),
                              trace=_trace)

    # reassemble layer-2 table by node id, then expand per-edge again
    h2n = np.empty((N + 1, E2), dtype=bfnp)
    pp = np.arange(P)
    jj = np.arange(NBLK)
    for c in range(CORES):
        oc = np.asarray(r1.results[c]["h2lo"]).reshape(NPC, E2)
        g = ((jj * CORES + c)[:, None] * P + pp[None, :]).reshape(-1)
        valid = g < N
        h2n[order[g[valid]]] = oc[valid]
    h2n[N] = bfnp(0.0)
    h2n[N, OUT_F:] = bfnp(-1e30)

    in2 = [{"he2": h2n[idx1[c]], "b2d": b2a} for c in range(CORES)]
    r2 = run_bass_kernel_spmd(nc2, in2, core_ids=list(range(CORES)),
                              trace=_trace)
    LAST_RESULT = (r0, r1, r2)

    out = np.zeros((N, OUT_F), dtype=np.float32)
    for c in range(CORES):
        oc = np.asarray(r2.results[c]["outp"]).reshape(NPC, OUT_F)
        g = ((jj * CORES + c)[:, None] * P + pp[None, :]).reshape(-1)
        valid = g < N
        out[order[g[valid]]] = oc[valid]
    return out
